# revision 7
# baseline (speedup 1.0000x reference)
"""Trainium2 Bass kernel for fake-quant (W8A8) linear: y = fq_tok(x) @ fq_ch(w).T + b.

Full shapes: x [4, 2048, 4096] f32, w [4096, 4096] f32, b [4096] f32.
Sharding over 8 cores: 2 token groups x 4 out-channel groups.
Per core: x_sh [4096, 4096], w_sh [1024, 4096], b_sh [1024] -> y_sh [4096, 1024].

Key idea: quantized values are integers in [-127, 127], exactly representable
in bf16, so the matmul runs on the PE array in bf16 (full rate) with fp32 PSUM
accumulation - numerically equivalent to the fp32 reference einsum on the
dequantized values.  Scales are applied in the fp32 epilogue.

Rounding: round-half-to-even via the fp32 magic-constant trick
(v + 1.5*2^23 rounds mantissa to integer; subtract again afterwards),
matching jnp.round.  Clipping to [-128, 127] is a no-op by construction
(|x|/s <= 127 when s = amax/127) so it is skipped.

v3: all 128x128 transposes go through the DMA XBAR transpose
(dma_start(transpose=True): out[p, kb, t] = in[t, kb*128+p], verified on HW)
so the PE executes ONLY the 2048 bf16 matmuls (N=512) per core = ~442 us
roofline.  The xbar ucode costs ~19ns per 16x128 tile ON THE ISSUING ENGINE
(~4.9us per full [128,4096] transpose), so each transpose is split in two
halves issued on ACT and SP; and because engine queues are strict FIFO, a
DMA issue queued behind an xbar's semaphore wait stalls the whole pipeline -
hence the load/quant job stream below is software-pipelined by 2 (tile j's
DMA issues before tile j-2's xbar) and the first two x tiles are interleaved
into the weight phase.  qwT is split into one tile per channel block so the
cb=0 matmuls only wait on the first 4 w-tiles.

Engine split per x tile: DVE amax/scale/recip + fp32 epilogue (~7.6us); ACT
the two rounding passes + half the xbar (~10us); SP the other xbar half +
DMA issues (~5us); PE the matmuls (13.7us, bottleneck).  Matmul blocks are
emitted two tiles behind quantization so the ~22us quantize chain pipelines
across two matmul blocks and the PE never starves (no HAM oscillation).
"""

from contextlib import ExitStack

import numpy as np

import concourse.bass as bass
import concourse.mybir as mybir
import concourse.tile as tile
from concourse import bacc

P = 128
MAGIC = 12582912.0  # 1.5 * 2**23
QMAX = 127.0
EPS = 1e-8

# full problem shapes (hardcoded per harness contract)
B, S, D_IN, D_OUT = 4, 2048, 4096, 4096
TOK = B * S  # 8192
TOK_GROUPS = 2
CH_GROUPS = 4
T_SH = TOK // TOK_GROUPS  # 4096 tokens per core
O_SH = D_OUT // CH_GROUPS  # 1024 channels per core


def build_nc(T, K, O, nch=512, lookahead=3):
    """Build the per-core Bass program: x[T,K], w[O,K], b[O] -> y[T,O]."""
    f32 = mybir.dt.float32
    bf16 = mybir.dt.bfloat16
    Copy = mybir.ActivationFunctionType.Copy
    Alu = mybir.AluOpType
    AxX = mybir.AxisListType.X

    assert T % P == 0 and K % P == 0 and O % P == 0
    TT, KB, WT = T // P, K // P, O // P
    NCH = min(nch, O)
    CB = O // NCH
    WPC = NCH // P  # w-row-tiles per channel block

    nc = bacc.Bacc("TRN2", target_bir_lowering=False, debug=False)
    x_ap = nc.dram_tensor("x", [T, K], f32, kind="ExternalInput").ap()
    w_ap = nc.dram_tensor("w", [O, K], f32, kind="ExternalInput").ap()
    b_ap = nc.dram_tensor("b", [O], f32, kind="ExternalInput").ap()
    y_ap = nc.dram_tensor("y", [T, O], f32, kind="ExternalOutput").ap()

    with tile.TileContext(nc) as tc, ExitStack() as ctx:
        singles = ctx.enter_context(tc.tile_pool(name="singles", bufs=1))
        bigf32 = ctx.enter_context(tc.tile_pool(name="bigf32", bufs=3))
        rnd = ctx.enter_context(tc.tile_pool(name="rnd", bufs=1))
        qpool = ctx.enter_context(tc.tile_pool(name="qpool", bufs=2))
        qtpool = ctx.enter_context(tc.tile_pool(name="qtpool", bufs=4))
        stats = ctx.enter_context(tc.tile_pool(name="stats", bufs=8))
        opool = ctx.enter_context(tc.tile_pool(name="opool", bufs=4))
        psum_pool = ctx.enter_context(tc.tile_pool(name="psum", bufs=4, space="PSUM"))
        dram = ctx.enter_context(tc.tile_pool(name="dram", bufs=1, space="DRAM"))

        # resident: transposed quantized weights (one tile per channel block
        # so cb=0 matmuls only depend on w-tiles 0..WPC-1), broadcast rows
        # qwTs[cb][f, j, k, c] = qw[(cb*WPC+j)*128 + c, k*128 + f]
        qwTs = [
            singles.tile([P, WPC, KB, P], bf16, name=f"qwT_{cb}") for cb in range(CB)
        ]
        sw_b = singles.tile([P, O], f32)
        bb_b = singles.tile([P, O], f32)
        sw_dram = dram.tile([O, 1], f32)

        HALF = K // 2
        HKB = KB // 2

        def xbar_transpose(q_t, qT_view):
            nc.sync.dma_start(out=qT_view, in_=q_t, transpose=True)

        def quantize(src_t, q_t, s_t, dve_round=False):
            # per-row amax -> scale (s_t), then round src*(1/s) to q_t (bf16)
            amax = stats.tile([P, 1], f32, tag="st", name="amax")
            nc.vector.reduce_max(
                out=amax, in_=src_t, axis=AxX, apply_absolute_value=True
            )
            nc.vector.tensor_scalar(
                out=s_t, in0=amax, scalar1=1.0 / QMAX, scalar2=EPS,
                op0=Alu.mult, op1=Alu.max,
            )
            r_t = stats.tile([P, 1], f32, tag="st", name="recip")
            nc.vector.reciprocal(out=r_t, in_=s_t)
            t_t = rnd.tile([P, K], f32, tag="rnd", name="t_round")
            # round pass 1 (scale is a per-partition pointer operand; the
            # Bacc event-semaphore pass legalizes its single-wait limit)
            if dve_round:
                # weight phase: DVE does the round so ACT (busy with the
                # interleaved x rounds during the ramp) is off the w path
                nc.vector.tensor_scalar(
                    out=t_t, in0=src_t, scalar1=r_t[:, 0:1], scalar2=MAGIC,
                    op0=Alu.mult, op1=Alu.add,
                )
            else:
                nc.scalar.activation(
                    out=t_t, in_=src_t, func=Copy, bias=MAGIC, scale=r_t[:, 0:1]
                )
            nc.scalar.activation(out=q_t, in_=t_t, func=Copy, bias=-MAGIC, scale=1.0)

        # ---- matmul block + epilogue for one token tile ----
        def matmul_block(tt, sx, qxT):
            # cb-outer: the first channel block's matmuls only need w-tiles
            # 0..WPC-1 (earlier ramp) and the epilogues spread out on DVE
            for cb in range(CB):
                psum = psum_pool.tile([P, NCH], f32, tag="psum", name=f"ps_{tt}_{cb}")
                for k in range(KB):
                    nc.tensor.matmul(
                        psum,
                        qxT[:, k, :],
                        qwTs[cb][:, :, k, :],
                        start=(k == 0),
                        stop=(k == KB - 1),
                    )
                o1 = opool.tile([P, NCH], f32, tag="o", name=f"o1_{tt}_{cb}")
                nc.vector.scalar_tensor_tensor(
                    out=o1, in0=psum, scalar=sx[:, 0:1],
                    in1=sw_b[:, cb * NCH : (cb + 1) * NCH],
                    op0=Alu.mult, op1=Alu.mult,
                )
                o2 = opool.tile([P, NCH], f32, tag="o", name=f"o2_{tt}_{cb}")
                nc.vector.tensor_add(
                    out=o2, in0=o1, in1=bb_b[:, cb * NCH : (cb + 1) * NCH]
                )
                nc.sync.dma_start(
                    out=y_ap[tt * P : (tt + 1) * P, cb * NCH : (cb + 1) * NCH], in_=o2
                )

        # ---- unified software-pipelined job stream ----
        # jobs: (kind, idx); loads are emitted 2 jobs ahead of the compute
        # work so DMA issues on SP never queue behind an xbar's semaphore
        # wait (strict-FIFO engine queues).
        pending = []  # quantized x tiles awaiting their matmul block
        loaded = {}

        def load_job(kind, idx):
            t = bigf32.tile([P, K], f32, tag="big", name=f"{kind}_{idx}")
            src = w_ap if kind == "w" else x_ap
            nc.sync.dma_start(out=t, in_=src[idx * P : (idx + 1) * P, :])
            loaded[(kind, idx)] = t

        def work_job(kind, idx):
            src_t = loaded.pop((kind, idx))
            if kind == "w":
                sw = stats.tile([P, 1], f32, tag="st", name=f"sw_{idx}")
                qw = qpool.tile([P, K], bf16, tag="q", name=f"qw_{idx}")
                quantize(src_t, qw, sw, dve_round=True)
                xbar_transpose(qw, qwTs[idx // WPC][:, idx % WPC])
                nc.sync.dma_start(out=sw_dram[idx * P : (idx + 1) * P, :], in_=sw)
            else:
                sx = stats.tile([P, 1], f32, tag="st", name=f"sx_{idx}")
                qx = qpool.tile([P, K], bf16, tag="q", name=f"qx_{idx}")
                quantize(src_t, qx, sx)
                qxT = qtpool.tile([P, KB, P], bf16)  # qxT[f,k,t] = qx[t,k*128+f]
                xbar_transpose(qx, qxT)
                pending.append((idx, sx, qxT))
                if len(pending) > lookahead:
                    matmul_block(*pending.pop(0))

        jobs = [("w", i) for i in range(WT)] + [("x", i) for i in range(TT)]
        DLOOK = 2  # DMA issue lookahead (jobs)
        for j in range(len(jobs) + DLOOK):
            if j < len(jobs):
                load_job(*jobs[j])
            if j >= DLOOK:
                work_job(*jobs[j - DLOOK])
            if j == WT + DLOOK:
                # all w-tiles quantized: broadcast scale & bias rows
                nc.sync.dma_start(
                    out=sw_b,
                    in_=bass.AP(
                        tensor=sw_dram.tensor, offset=sw_dram.offset,
                        ap=[[0, P], [1, O]],
                    ),
                )
                nc.sync.dma_start(
                    out=bb_b,
                    in_=bass.AP(
                        tensor=b_ap.tensor, offset=b_ap.offset, ap=[[0, P], [1, O]]
                    ),
                )
        for args in pending:
            matmul_block(*args)
    nc.compile()
    return nc


_cached_nc = None


def _get_nc():
    global _cached_nc
    if _cached_nc is None:
        _cached_nc = build_nc(T_SH, D_IN, O_SH)
    return _cached_nc


def kernel(x: np.ndarray, w: np.ndarray, b: np.ndarray, _trace=False):
    from concourse.bass_utils import run_bass_kernel_spmd

    assert x.shape == (B, S, D_IN) and w.shape == (D_OUT, D_IN) and b.shape == (D_OUT,)
    x2 = np.ascontiguousarray(x.reshape(TOK, D_IN), dtype=np.float32)
    w2 = np.ascontiguousarray(w, dtype=np.float32)
    b2 = np.ascontiguousarray(b, dtype=np.float32)

    in_maps = []
    for core in range(8):
        tg, cg = divmod(core, CH_GROUPS)
        in_maps.append(
            {
                "x": np.ascontiguousarray(x2[tg * T_SH : (tg + 1) * T_SH]),
                "w": np.ascontiguousarray(w2[cg * O_SH : (cg + 1) * O_SH]),
                "b": np.ascontiguousarray(b2[cg * O_SH : (cg + 1) * O_SH]),
            }
        )

    nc = _get_nc()
    res = run_bass_kernel_spmd(nc, in_maps, core_ids=list(range(8)), trace=_trace)

    y = np.empty((TOK, D_OUT), dtype=np.float32)
    for core in range(8):
        tg, cg = divmod(core, CH_GROUPS)
        y[tg * T_SH : (tg + 1) * T_SH, cg * O_SH : (cg + 1) * O_SH] = res.results[
            core
        ]["y"]
    if _trace:
        kernel._last_results = res
    return y.reshape(B, S, D_OUT)


# revision 8
# speedup vs baseline: 1.2935x; 1.2935x over previous
"""Trainium2 Bass kernel for fake-quant (W8A8) linear: y = fq_tok(x) @ fq_ch(w).T + b.

Full shapes: x [4, 2048, 4096] f32, w [4096, 4096] f32, b [4096] f32.
Sharding over 8 cores: 2 token groups x 4 out-channel groups.
Per core: x_sh [4096, 4096], w_sh [1024, 4096], b_sh [1024] -> y_sh [4096, 1024].

Key idea: quantized values are integers in [-127, 127], exactly representable
in bf16, so the matmul runs on the PE array in bf16 (full rate) with fp32 PSUM
accumulation - numerically equivalent to the fp32 reference einsum on the
dequantized values.  Scales are applied in the fp32 epilogue.

Rounding: round-half-to-even via the fp32 magic-constant trick
(v + 1.5*2^23 rounds mantissa to integer; subtract again afterwards),
matching jnp.round.  Clipping to [-128, 127] is a no-op by construction.

v4 transpose strategy (measured on HW):
- The DMA XBAR transpose moves ~66 GB/s (256B packet per 16x128 tile per
  SDMA engine): a full [128,4096] bf16 transpose takes ~15us of DMA-ring
  time, and the XBAR CANNOT run two transposes concurrently (issuing on
  both HWDGE rings corrupts data - verified), so transposes are a single
  serial ~15us/tile resource.  PE transpose-mode matmuls cost ~72ns each
  in a warm stream but eat PE throughput.
- Hybrid split per x tile: 24 k-blocks via one XBAR instruction (~11.3us,
  SP ring) + 8 k-blocks via PE transposes (~0.6us PE + one DVE PSUM copy).
- The two HWDGE rings are FIFO per issuing engine, so loads (ACT ring) are
  isolated from the transpose+store stream (SP ring).
- Weight tiles are transposed entirely on the PE during the ramp (PE is
  idle there anyway; XBAR would serialize 8x15us = 120us).

Engine split per x tile: DVE amax/scale/recip + psum-copy + fp32 epilogue
(~8.7us); ACT two rounding passes + load issues (~8.7us); SP xbar ucode +
store issues (~6us); PE matmuls + 8 transposes (~14.4us, bottleneck).
The load/quant job stream is software-pipelined by 2 and matmul blocks are
emitted `lookahead` tiles behind quantization so the quantize chain
pipelines across matmul blocks and the PE never starves.
"""

from contextlib import ExitStack

import numpy as np

import concourse.bass as bass
import concourse.mybir as mybir
import concourse.tile as tile
from concourse import bacc
from concourse.masks import make_identity

P = 128
MAGIC = 12582912.0  # 1.5 * 2**23
QMAX = 127.0
EPS = 1e-8

# full problem shapes (hardcoded per harness contract)
B, S, D_IN, D_OUT = 4, 2048, 4096, 4096
TOK = B * S  # 8192
TOK_GROUPS = 2
CH_GROUPS = 4
T_SH = TOK // TOK_GROUPS  # 4096 tokens per core
O_SH = D_OUT // CH_GROUPS  # 1024 channels per core


def build_nc(T, K, O, nch=512, lookahead=3, pe_kb=8):
    """Build the per-core Bass program: x[T,K], w[O,K], b[O] -> y[T,O]."""
    f32 = mybir.dt.float32
    bf16 = mybir.dt.bfloat16
    Copy = mybir.ActivationFunctionType.Copy
    Alu = mybir.AluOpType
    AxX = mybir.AxisListType.X

    assert T % P == 0 and K % P == 0 and O % P == 0
    TT, KB, WT = T // P, K // P, O // P
    NCH = min(nch, O)
    CB = O // NCH
    WPC = NCH // P  # w-row-tiles per channel block
    XKB = KB - pe_kb  # k-blocks transposed by the XBAR per x tile
    TG = 8  # k-blocks per PE-transpose psum group (8*128 bf16 = one bank)

    nc = bacc.Bacc("TRN2", target_bir_lowering=False, debug=False)
    x_ap = nc.dram_tensor("x", [T, K], f32, kind="ExternalInput").ap()
    w_ap = nc.dram_tensor("w", [O, K], f32, kind="ExternalInput").ap()
    b_ap = nc.dram_tensor("b", [O], f32, kind="ExternalInput").ap()
    y_ap = nc.dram_tensor("y", [T, O], f32, kind="ExternalOutput").ap()

    with tile.TileContext(nc) as tc, ExitStack() as ctx:
        singles = ctx.enter_context(tc.tile_pool(name="singles", bufs=1))
        bigf32 = ctx.enter_context(tc.tile_pool(name="bigf32", bufs=3))
        rnd = ctx.enter_context(tc.tile_pool(name="rnd", bufs=1))
        qpool = ctx.enter_context(tc.tile_pool(name="qpool", bufs=2))
        qtpool = ctx.enter_context(tc.tile_pool(name="qtpool", bufs=4))
        stats = ctx.enter_context(tc.tile_pool(name="stats", bufs=12))
        opool = ctx.enter_context(tc.tile_pool(name="opool", bufs=4))
        psum_pool = ctx.enter_context(tc.tile_pool(name="psum", bufs=4, space="PSUM"))
        tpsum = ctx.enter_context(tc.tile_pool(name="tpsum", bufs=3, space="PSUM"))
        dram = ctx.enter_context(tc.tile_pool(name="dram", bufs=1, space="DRAM"))

        # resident: transposed quantized weights (one tile per channel block
        # so cb=0 matmuls only depend on w-tiles 0..WPC-1), broadcast rows
        # qwTs[cb][f, j, k, c] = qw[(cb*WPC+j)*128 + c, k*128 + f]
        qwTs = [
            singles.tile([P, WPC, KB, P], bf16, name=f"qwT_{cb}") for cb in range(CB)
        ]
        sw_b = singles.tile([P, O], f32)
        bb_b = singles.tile([P, O], f32)
        sw_dram = dram.tile([O, 1], f32)
        ident = singles.tile([P, P], bf16)
        make_identity(nc, ident)

        def pe_transpose(q_view, dst, tag, kb0, nkb, copy_engines=("v",)):
            # q_view [P, nkb*P] bf16 (k-blocks kb0..kb0+nkb-1 of the source)
            # -> dst[:, kb0:kb0+nkb, :] via PE transpose + PSUM copy
            for g0 in range(0, nkb, TG):
                g = min(TG, nkb - g0)
                tp = tpsum.tile([P, g, P], bf16, tag="tp", name=f"tp_{tag}_{g0}")
                for j in range(g):
                    kb = kb0 + g0 + j
                    nc.tensor.transpose(
                        tp[:, j, :], q_view[:, kb * P : (kb + 1) * P], ident
                    )
                eng = copy_engines[(g0 // TG) % len(copy_engines)]
                if eng == "v":
                    nc.vector.tensor_copy(
                        out=dst[:, kb0 + g0 : kb0 + g0 + g, :], in_=tp
                    )
                else:
                    nc.scalar.activation(
                        out=dst[:, kb0 + g0 : kb0 + g0 + g, :], in_=tp, func=Copy
                    )

        def quantize(src_t, q_t, s_t, dve_round=False):
            # per-row amax -> scale (s_t), then round src*(1/s) to q_t (bf16)
            amax = stats.tile([P, 1], f32, tag="st", name="amax")
            nc.vector.reduce_max(
                out=amax, in_=src_t, axis=AxX, apply_absolute_value=True
            )
            nc.vector.tensor_scalar(
                out=s_t, in0=amax, scalar1=1.0 / QMAX, scalar2=EPS,
                op0=Alu.mult, op1=Alu.max,
            )
            r_t = stats.tile([P, 1], f32, tag="st", name="recip")
            nc.vector.reciprocal(out=r_t, in_=s_t)
            t_t = rnd.tile([P, K], f32, tag="rnd", name="t_round")
            # round pass 1 (scale is a per-partition pointer operand; the
            # Bacc event-semaphore pass legalizes its single-wait limit)
            if dve_round:
                # weight phase: DVE does the round so ACT (doing round2 +
                # interleaved x rounds during the ramp) is off the w path
                nc.vector.tensor_scalar(
                    out=t_t, in0=src_t, scalar1=r_t[:, 0:1], scalar2=MAGIC,
                    op0=Alu.mult, op1=Alu.add,
                )
            else:
                nc.scalar.activation(
                    out=t_t, in_=src_t, func=Copy, bias=MAGIC, scale=r_t[:, 0:1]
                )
            nc.scalar.activation(out=q_t, in_=t_t, func=Copy, bias=-MAGIC, scale=1.0)

        # ---- matmul block + epilogue for one token tile ----
        def matmul_block(tt, sx, qxT):
            # cb-outer: the first channel block's matmuls only need w-tiles
            # 0..WPC-1 (earlier ramp) and the epilogues spread out on DVE
            for cb in range(CB):
                psum = psum_pool.tile([P, NCH], f32, tag="psum", name=f"ps_{tt}_{cb}")
                for k in range(KB):
                    nc.tensor.matmul(
                        psum,
                        qxT[:, k, :],
                        qwTs[cb][:, :, k, :],
                        start=(k == 0),
                        stop=(k == KB - 1),
                    )
                o1 = opool.tile([P, NCH], f32, tag="o", name=f"o1_{tt}_{cb}")
                nc.vector.scalar_tensor_tensor(
                    out=o1, in0=psum, scalar=sx[:, 0:1],
                    in1=sw_b[:, cb * NCH : (cb + 1) * NCH],
                    op0=Alu.mult, op1=Alu.mult,
                )
                o2 = opool.tile([P, NCH], f32, tag="o", name=f"o2_{tt}_{cb}")
                nc.vector.tensor_add(
                    out=o2, in0=o1, in1=bb_b[:, cb * NCH : (cb + 1) * NCH]
                )
                nc.sync.dma_start(
                    out=y_ap[tt * P : (tt + 1) * P, cb * NCH : (cb + 1) * NCH], in_=o2
                )

        # ---- unified software-pipelined job stream ----
        # loads are emitted DLOOK jobs ahead of the compute work; loads go on
        # the ACT HWDGE ring, xbar transposes + stores on the SP ring (the
        # rings are FIFO per issuing engine, and the XBAR must never run two
        # transposes concurrently - single ring serializes them).
        pending = []  # quantized x tiles awaiting their matmul block
        loaded = {}
        n_ramp_x = 2  # x tiles quantized inside the w phase (PE-transposed)

        def load_job(kind, idx):
            t = bigf32.tile([P, K], f32, tag="big", name=f"{kind}_{idx}")
            src = w_ap if kind == "w" else x_ap
            nc.scalar.dma_start(out=t, in_=src[idx * P : (idx + 1) * P, :])
            loaded[(kind, idx)] = t

        def work_job(kind, idx):
            src_t = loaded.pop((kind, idx))
            if kind == "w":
                sw = stats.tile([P, 1], f32, tag="st", name=f"sw_{idx}")
                qw = qpool.tile([P, K], bf16, tag="q", name=f"qw_{idx}")
                quantize(src_t, qw, sw, dve_round=True)
                pe_transpose(
                    qw, qwTs[idx // WPC][:, idx % WPC], f"w{idx}", 0, KB,
                    copy_engines=("v", "s"),
                )
                nc.sync.dma_start(out=sw_dram[idx * P : (idx + 1) * P, :], in_=sw)
            else:
                sx = stats.tile([P, 1], f32, tag="st", name=f"sx_{idx}")
                qx = qpool.tile([P, K], bf16, tag="q", name=f"qx_{idx}")
                quantize(src_t, qx, sx)
                qxT = qtpool.tile([P, KB, P], bf16)  # qxT[f,k,t] = qx[t,k*128+f]
                if idx < n_ramp_x:
                    # ramp: PE is idle, transpose everything on it
                    pe_transpose(qx, qxT, f"x{idx}", 0, KB, copy_engines=("v", "s"))
                else:
                    if XKB:
                        nc.sync.dma_start(
                            out=qxT[:, :XKB], in_=qx[:, : XKB * P], transpose=True
                        )
                    if pe_kb:
                        pe_transpose(qx, qxT, f"x{idx}", XKB, pe_kb)
                pending.append((idx, sx, qxT))
                if len(pending) > lookahead:
                    matmul_block(*pending.pop(0))

        jobs = (
            [("w", i) for i in range(WPC)]
            + [("x", 0)]
            + [("w", i) for i in range(WPC, WT)]
            + [("x", 1)]
            + [("x", i) for i in range(2, TT)]
        )
        last_w_j = max(j for j, (k, _) in enumerate(jobs) if k == "w")
        DLOOK = 2  # DMA issue lookahead (jobs)
        for j in range(len(jobs) + DLOOK):
            if j < len(jobs):
                load_job(*jobs[j])
            if j >= DLOOK:
                work_job(*jobs[j - DLOOK])
            if j == last_w_j + DLOOK:
                # all w-tiles quantized: broadcast scale & bias rows
                nc.sync.dma_start(
                    out=sw_b,
                    in_=bass.AP(
                        tensor=sw_dram.tensor, offset=sw_dram.offset,
                        ap=[[0, P], [1, O]],
                    ),
                )
                nc.sync.dma_start(
                    out=bb_b,
                    in_=bass.AP(
                        tensor=b_ap.tensor, offset=b_ap.offset, ap=[[0, P], [1, O]]
                    ),
                )
        for args in pending:
            matmul_block(*args)
    nc.compile()
    return nc


_cached_nc = None


def _get_nc():
    global _cached_nc
    if _cached_nc is None:
        _cached_nc = build_nc(T_SH, D_IN, O_SH)
    return _cached_nc


def kernel(x: np.ndarray, w: np.ndarray, b: np.ndarray, _trace=False):
    from concourse.bass_utils import run_bass_kernel_spmd

    assert x.shape == (B, S, D_IN) and w.shape == (D_OUT, D_IN) and b.shape == (D_OUT,)
    x2 = np.ascontiguousarray(x.reshape(TOK, D_IN), dtype=np.float32)
    w2 = np.ascontiguousarray(w, dtype=np.float32)
    b2 = np.ascontiguousarray(b, dtype=np.float32)

    in_maps = []
    for core in range(8):
        tg, cg = divmod(core, CH_GROUPS)
        in_maps.append(
            {
                "x": np.ascontiguousarray(x2[tg * T_SH : (tg + 1) * T_SH]),
                "w": np.ascontiguousarray(w2[cg * O_SH : (cg + 1) * O_SH]),
                "b": np.ascontiguousarray(b2[cg * O_SH : (cg + 1) * O_SH]),
            }
        )

    nc = _get_nc()
    res = run_bass_kernel_spmd(nc, in_maps, core_ids=list(range(8)), trace=_trace)

    y = np.empty((TOK, D_OUT), dtype=np.float32)
    for core in range(8):
        tg, cg = divmod(core, CH_GROUPS)
        y[tg * T_SH : (tg + 1) * T_SH, cg * O_SH : (cg + 1) * O_SH] = res.results[
            core
        ]["y"]
    if _trace:
        kernel._last_results = res
    return y.reshape(B, S, D_OUT)
